# revision 7
# baseline (speedup 1.0000x reference)
"""Trainium2 Bass kernel for gumbel hard-attention (sparse_attention), v2.

Forward value of hard gumbel-softmax is one_hot(argmax(mask(q@kT)*scale+gum)),
so y[b,h,q,:] = v[b,h,argmax_k,:]; softmax is monotone, argmax over logits.

Precision: the argmax needs ~2^-17 logit fidelity (one flipped row costs
~1e-2 rel err). q,k and q@kT are computed with fp16 hi/lo split arithmetic
(error ~2^-21): x and w_qk are split on the host into fp16 (hi, lo) pairs;
q^T,k^T = 18 accumulated fp16 matmuls (6 hi*hi + 12 cross); the qk matmul
runs as hh (K=64) plus one concatenated cross matmul [qh;ql]@[kl;kh] (K=128).
v and c_proj run in plain fp16.

Per block-row: DVE does a fused (att+gum, running-max) pass via
tensor_tensor_reduce; GpSimd extracts the argmax position with
scalar_tensor_tensor sum((apg==M)*iota). The v-gather is a SWDGE
dma_gather (transpose mode) from a DRAM scratch copy of v, yielding y^T
directly; indices are re-wrapped to the [16]-partition layout via a tiny
DRAM bounce and replicated to 128 partitions with a matmul.

Gumbel streams on the DVE-issued HWDGE queue so it prefetches while the
SP queue loads x/w; x/w/out use the SP queue.

Sharding: 24 (b,h) over 8 cores -> 3 heads/core; partial c_proj summed on
host. Causal mask is folded into the gumbel diagonal blocks host-side.
"""

import sys

for p in ("/opt/trn_rl_repo",):
    if p not in sys.path:
        sys.path.insert(0, p)

import numpy as np

import concourse.bacc as bacc
import concourse.dve_ops as dve_ops
import concourse.mybir as mybir
from concourse.bass_utils import run_bass_kernel_spmd
from concourse.dve_spec import (AluOp, C0, C1, Idx, One, Spec, Src0, Src1,
                                Zero, eq, scan, select)
from concourse.tile import TileContext


def _ref_add_argmax(in0, in1, c0, c1, c2):
    """out = where(z==runmax, idx, -1) candidates of z = in0+in1;
    accum = max(out) = argmax position (last max wins)."""
    z = in0.astype(np.float32) + in1.astype(np.float32)
    r = np.maximum.accumulate(z, axis=-1)
    ii = np.arange(z.shape[-1], dtype=np.float32)[None, :]
    cand = np.where(z == r, np.broadcast_to(ii, z.shape), np.float32(-1.0))
    return cand, cand.max(axis=-1, keepdims=True)


def _register_add_argmax():
    if "ADD_ARGMAX_ANT" in dve_ops._SUB_OPCODE_FOR_NAME:
        return [op for op in dve_ops.OPS if op.name == "ADD_ARGMAX_ANT"][0]
    z = Src0 + Src1
    r = scan(AluOp.MAX, z)
    cand = select(eq(z, r), Idx, Zero - One)
    spec = Spec(body=cand, accum=AluOp.MAX, reference=_ref_add_argmax)
    op = dve_ops.DveOp(
        "ADD_ARGMAX_ANT", spec, subdim=False,
        uops_sha={"v3": "d4bb101012599987"})
    dve_ops.OPS.append(op)
    dve_ops.CUSTOM_DVE_SPECS[op.name] = spec
    dve_ops._SUB_OPCODE_FOR_NAME[op.name] = (
        max(dve_ops._SUB_OPCODE_FOR_NAME.values()) + 1)
    return op


_ADD_ARGMAX = _register_add_argmax()

F32 = mybir.dt.float32
BF16 = mybir.dt.bfloat16
FP16 = mybir.dt.float16
I16 = mybir.dt.int16

T = 2048          # sequence length
C = 768           # model dim
HPC = 3           # heads per core
HD = 64           # head dim
NB = T // 128     # 16 q-blocks
EC = C // 128     # 6 contraction chunks
NEG = -1e9
QKC = 2 * HPC * HD   # 384 = packed q|k head-dim columns
VC = HPC * HD        # 192


def build_program():
    nc = bacc.Bacc(target_bir_lowering=False, trn_type="TRN2")

    xhT = nc.dram_tensor("xhT", [C, T], FP16, kind="ExternalInput")
    xlT = nc.dram_tensor("xlT", [C, T], FP16, kind="ExternalInput")
    wqkh = nc.dram_tensor("wqkh", [C, QKC], FP16, kind="ExternalInput")
    wqkl = nc.dram_tensor("wqkl", [C, QKC], FP16, kind="ExternalInput")
    wvh = nc.dram_tensor("wvh", [C, VC], FP16, kind="ExternalInput")
    wpd = nc.dram_tensor("wpd", [VC, C], FP16, kind="ExternalInput")
    gum = nc.dram_tensor("gum", [HPC, T, T], F32, kind="ExternalInput")
    id16 = nc.dram_tensor("id16", [128, 128], FP16, kind="ExternalInput")
    iotar = nc.dram_tensor("iotar", [128, T], FP16, kind="ExternalInput")
    repl16 = nc.dram_tensor("repl16", [16, 128], F32, kind="ExternalInput")

    # v rows in gather layout: row t = [v_h0(64) pad(64) v_h1 pad v_h2 pad]
    v_scr = nc.dram_tensor("v_scr", [NB, 128, HPC, 2, HD], FP16, kind="Internal")
    # wrapped argmax indices per half: [head][half][r][blk'][a]
    # (query q = 1024*half + 128*blk' + 16*a + r)
    idx_scr = nc.dram_tensor("idx_scr", [HPC, 2, 16, 8, 8], F32, kind="Internal")
    # quarter-granularity variant for the final head's tail
    idx_scrq = nc.dram_tensor("idx_scrq", [4, 16, 4, 8], F32, kind="Internal")

    out = nc.dram_tensor("out", [T, C], BF16, kind="ExternalOutput")

    with TileContext(nc) as tc:
        with (
            tc.tile_pool(name="const", bufs=1) as cpool,
            tc.tile_pool(name="big", bufs=1) as bigpool,
            tc.tile_pool(name="gumld", bufs=8) as gpool,
            tc.tile_pool(name="scr", bufs=1) as scrpool,
            tc.tile_pool(name="stat", bufs=4) as spool,
            tc.tile_pool(name="io", bufs=3) as iopool,
            tc.tile_pool(name="psqk", bufs=1, space="PSUM") as pqk_pool,   # 4 banks
            tc.tile_pool(name="psmm", bufs=2, space="PSUM") as pmm,       # 2 banks
            tc.tile_pool(name="pstr", bufs=2, space="PSUM") as ptr16,     # 1 bank
        ):
            id16_sb = cpool.tile([128, 128], FP16, tag="id16")
            repl_sb = cpool.tile([16, 128], F32, tag="repl16")
            wqkh_sb = bigpool.tile([128, EC, QKC], FP16, tag="wqkh")
            wqkl_sb = bigpool.tile([128, EC, QKC], FP16, tag="wqkl")
            wvh_sb = bigpool.tile([128, EC, VC], FP16, tag="wvh")
            nc.sync.dma_start(wqkh_sb, wqkh.rearrange("(e p) c -> p e c", p=128))
            nc.sync.dma_start(wqkl_sb, wqkl.rearrange("(e p) c -> p e c", p=128))
            wpH = bigpool.tile([64, HPC, C], FP16, tag="wpH")
            wpH2 = bigpool.tile([128, C], FP16, tag="wpH2")

            # ---- Phase 1: q^T/k^T hi+lo stacks and v^T, streaming x by t4
            # qstack: partitions 0-63 = qh^T(head), 64-127 = ql^T(head)
            # kstack: partitions 0-63 = kl^T(head), 64-127 = kh^T(head)
            # khT2:   kh^T duplicated at base partition 0 (hh matmul rhs)
            qstack = bigpool.tile([128, HPC, T], FP16, tag="qstack")
            kstack = bigpool.tile([128, HPC, T], FP16, tag="kstack")
            khT2 = bigpool.tile([64, HPC, T], FP16, tag="khT2")
            vtA = bigpool.tile([128, T], FP16, tag="vtA")   # v^T heads 0,1
            vtB = bigpool.tile([64, T], FP16, tag="vtB")    # v^T head 2

            xhT_r = xhT.rearrange("(e p) t -> p e t", p=128)
            xlT_r = xlT.rearrange("(e p) t -> p e t", p=128)
            xpool_cm = tc.tile_pool(name="xT", bufs=1)
            xpool = xpool_cm.__enter__()
            xh_sb = xpool.tile([128, EC, T], FP16, tag="xh")
            xl_sb = xpool.tile([128, EC, T], FP16, tag="xl")
            for ec in range(EC):
                nc.sync.dma_start(xh_sb[:, ec, :], xhT_r[:, ec, :])
                nc.sync.dma_start(xl_sb[:, ec, :], xlT_r[:, ec, :])
            nc.sync.dma_start(wvh_sb, wvh.rearrange("(e p) c -> p e c", p=128))
            nc.sync.dma_start(id16_sb, id16[:, :])
            nc.sync.dma_start(repl_sb, repl16[:, :])
            nc.sync.dma_start(wpH, wpd.rearrange("(h p) c -> p h c", p=64))
            nc.sync.dma_start(wpH2, wpd[0:128, :])
            spl_cm = tc.tile_pool(name="split", bufs=2)
            spl = spl_cm.__enter__()

            v_nat = bigpool.tile([128, NB, VC], FP16, tag="v_nat")
            ystack = bigpool.tile([128, T], FP16, tag="ystack")  # h0 | h1
            y1 = bigpool.tile([128, T], FP16, tag="y1")
            y2 = bigpool.tile([128, T], FP16, tag="y2")
            v_flat = v_scr.rearrange("b p h s e -> (b p) (h s e)")

            def _splits(h, ts, pq):
                hi16 = spl.tile([128, 512], FP16, tag="hi")
                lo16 = spl.tile([128, 512], FP16, tag="lo")
                nc.vector.tensor_copy(hi16, pq[:, 0:512])
                nc.vector.tensor_tensor(
                    lo16, pq[:, 0:512], hi16, mybir.AluOpType.subtract)
                nc.gpsimd.tensor_copy(qstack[0:64, h, ts], hi16[0:64, :])
                nc.gpsimd.tensor_copy(qstack[64:128, h, ts], lo16[0:64, :])
                nc.gpsimd.tensor_copy(kstack[64:128, h, ts], hi16[64:128, :])
                nc.gpsimd.tensor_copy(khT2[:, h, ts], hi16[64:128, :])
                nc.gpsimd.tensor_copy(kstack[0:64, h, ts], lo16[64:128, :])

            _JWX = ([(0, 0)] * 6 + [(1, 0)] * 6 + [(0, 1)] * 6,)[0]

            def _piece_mm(pq, h, t4, j, j0, j1):
                wi, xi = _JWX[j]
                wsb = (wqkh_sb, wqkl_sb)[wi]
                xsb = (xh_sb, xl_sb)[xi]
                ec = j % 6
                ts = slice(t4 * 512, (t4 + 1) * 512)
                nc.tensor.matmul(
                    pq[:, 0:512],
                    wsb[:, ec, h * 128:(h + 1) * 128],
                    xsb[:, ec, ts],
                    start=(j == j0), stop=(j == j1))

            piece_ps = {}

            def piece_half(h, t4, part):
                """9 of the 18 accumulating qkv matmuls for (head, t4)."""
                ts = slice(t4 * 512, (t4 + 1) * 512)
                if part == 0:
                    piece_ps[(h, t4)] = pmm.tile(
                        [128, 512], F32, tag="mm", name=f"pp_{h}_{t4}")
                pq = piece_ps[(h, t4)]
                for j in range(part * 9, part * 9 + 9):
                    _piece_mm(pq, h, t4, j, 0, 17)
                if part == 1:
                    _splits(h, ts, pq)

            def piece0():
                """Head-0 piece: 4 concurrent ec-outer groups so the matmuls
                track the x chunk arrivals."""
                big = pqk_pool.tile([128, 2048], F32, tag="pqk", name="pgb")
                gps = [pmm.tile([128, 512], F32, tag="mm", name="pg0"),
                       pmm.tile([128, 512], F32, tag="mm", name="pg1"),
                       big[:, 0:512], big[:, 512:1024]]
                combos = ((wqkh_sb, xh_sb), (wqkl_sb, xh_sb), (wqkh_sb, xl_sb))
                for ec in range(EC):
                    for t4 in range(4):
                        ts = slice(t4 * 512, (t4 + 1) * 512)
                        for ci, (wsb, xsb) in enumerate(combos):
                            nc.tensor.matmul(
                                gps[t4][:, 0:512],
                                wsb[:, ec, 0:128],
                                xsb[:, ec, ts],
                                start=(ec == 0 and ci == 0),
                                stop=(ec == EC - 1 and ci == 2))
                for t4 in range(4):
                    _splits(0, slice(t4 * 512, (t4 + 1) * 512), gps[t4])

            def v_group(t4, pc):
                ts = slice(t4 * 512, (t4 + 1) * 512)
                dst, cn = ((vtA, 128), (vtB, 64))[pc]
                pv = pmm.tile([128, 512], F32, tag="mm")
                for ec in range(EC):
                    nc.tensor.matmul(
                        pv[:cn, 0:512],
                        wvh_sb[:, ec, pc * 128:pc * 128 + cn],
                        xh_sb[:, ec, ts],
                        start=(ec == 0), stop=(ec == EC - 1))
                nc.vector.tensor_copy(dst[:, ts], pv[:cn, 0:512])

            def vnat_half(g):
                for tb in range(g * 8, g * 8 + 8):
                    bs = slice(tb * 128, (tb + 1) * 128)
                    pv = ptr16.tile([128, 512], FP16, tag="tr16")
                    nc.tensor.transpose(pv[:, 0:128], vtA[:, bs], id16_sb)
                    nc.tensor.transpose(pv[:, 128:192], vtB[:, bs],
                                        id16_sb[0:64, 0:64])
                    nc.vector.tensor_copy(v_nat[:, tb, :], pv[:, 0:192])

            def vscr_write():
                for hh in range(HPC):
                    nc.sync.dma_start(
                        v_scr[:, :, hh, 0, :].rearrange("b p e -> p b e"),
                        v_nat[:, :, hh * HD:(hh + 1) * HD])

            def half_gather(h, idxall, half):
                """Wrap+gather queries [half*1024, (half+1)*1024)."""
                n2 = T // 2
                rb16 = spool.tile([16, 64], F32, tag="rb16")
                nc.sync.dma_start(
                    rb16, idx_scr[h, half].rearrange("r b a -> r (b a)"))
                pidx = pmm.tile([128, 512], F32, tag="mm")
                nc.tensor.matmul(pidx[:, 0:64], repl_sb, rb16,
                                 start=True, stop=True)
                idx16 = spool.tile([128, 64], I16, tag="idx16")
                nc.vector.tensor_copy(idx16, pidx[:, 0:64])
                ydst = (ystack, y1, y2)[h]
                hsl = slice(half * n2, (half + 1) * n2)
                nc.gpsimd.dma_gather(
                    out_ap=ydst[:, hsl].rearrange("p (s t) -> p s t", s=1),
                    in_ap=v_flat[:, h * 128:(h + 1) * 128],
                    idxs_ap=idx16,
                    num_idxs=n2,
                    num_idxs_reg=n2,
                    elem_size=128,
                    elem_step=2 * HPC * HD,
                    single_packet=False,
                    transpose=True)
                if h == 1:
                    nc.gpsimd.tensor_copy(ystack[64:128, hsl], y1[0:64, hsl])

            def quarter_gather(h, idxall, qt):
                """Wrap+gather queries [qt*512, (qt+1)*512) (h=2 tail)."""
                n4 = T // 4
                rb16 = spool.tile([16, 32], F32, tag="rb16")
                nc.sync.dma_start(
                    rb16, idx_scrq[qt].rearrange("r b a -> r (b a)"))
                pidx = pmm.tile([128, 512], F32, tag="mm")
                nc.tensor.matmul(pidx[:, 0:32], repl_sb, rb16,
                                 start=True, stop=True)
                idx16 = spool.tile([128, 32], I16, tag="idx16")
                nc.vector.tensor_copy(idx16, pidx[:, 0:32])
                ydst = (ystack, y1, y2)[h]
                hsl = slice(qt * n4, (qt + 1) * n4)
                nc.gpsimd.dma_gather(
                    out_ap=ydst[:, hsl].rearrange("p (s t) -> p s t", s=1),
                    in_ap=v_flat[:, h * 128:(h + 1) * 128],
                    idxs_ap=idx16,
                    num_idxs=n4,
                    num_idxs_reg=n4,
                    elem_size=128,
                    elem_step=2 * HPC * HD,
                    single_packet=False,
                    transpose=True)

            def proj_tbs(tbs):
                for tb in tbs:
                    bs = slice(tb * 128, (tb + 1) * 128)
                    ost = iopool.tile([128, C], BF16, tag="ost")
                    for fc in (0, 384):
                        po = pmm.tile([128, 512], F32, tag="mm", name="po")
                        nc.tensor.matmul(
                            po[:, 0:384], ystack[:, bs],
                            wpH2[:, fc:fc + 384], start=True, stop=False)
                        nc.tensor.matmul(
                            po[:, 0:384], y2[0:64, bs],
                            wpH[:, 2, fc:fc + 384], start=False, stop=True)
                        nc.scalar.copy(ost[:, fc:fc + 384], po[:, 0:384])
                    nc.sync.dma_start(out[bs, :], ost)

            def pump(h, p, idxall):
                """Emit background PE work after head h's p-th processed
                block (blocks run big-to-small: block = 15 - p)."""
                if h == 0:
                    if p < 8:
                        v_group(p // 2, p % 2)
                    if 4 <= p < 12:
                        piece_half(1, (p - 4) // 2, (p - 4) % 2)
                    if p == 12:
                        vnat_half(0)
                    elif p == 13:
                        vnat_half(1)
                    elif p == 14:
                        vscr_write()
                    elif p == 15:
                        half_gather(0, idxall, 1)
                elif h == 1:
                    if p < 8:
                        piece_half(2, p // 2, p % 2)
                    if p == 8:
                        half_gather(1, idxall, 1)
                else:
                    if p == 7:
                        half_gather(2, idxall, 1)
                    elif 8 <= p < 12:
                        proj_tbs([p])
                    elif p == 12:
                        quarter_gather(2, idxall, 1)
                        proj_tbs([12])
                    elif p in (13, 14):
                        proj_tbs([p, p - 9])
                    elif p == 15:
                        proj_tbs([15, 6, 7])

            piece0()
            for h in range(HPC):
                idxall = spool.tile([128, NB], F32, tag="idxall")
                for p in range(NB):
                    i = NB - 1 - p
                    W = (i + 1) * 128
                    qs = slice(i * 128, (i + 1) * 128)
                    gt = gpool.tile([128, T], F32, tag="gum")
                    nc.scalar.dma_start(gt[:, :W], gum[h, qs, 0:W])
                    pa = pqk_pool.tile([128, 2048], F32, tag="pqk")
                    for s in range(0, W, 512):
                        sw = min(512, W - s)
                        ks = slice(s, s + sw)
                        nc.tensor.matmul(
                            pa[:, s:s + sw],
                            qstack[0:64, h, qs], khT2[:, h, ks],
                            start=True, stop=False)
                        nc.tensor.matmul(
                            pa[:, s:s + sw],
                            qstack[:, h, qs], kstack[:, h, ks],
                            start=False, stop=True)
                    scr = scrpool.tile([128, T], FP16, tag="scr")
                    nc.vector._custom_dve(
                        _ADD_ARGMAX,
                        out=scr[:, :W],
                        in0=pa[:, :W],
                        in1=gt[:, :W],
                        accum_out=idxall[:, i:i + 1])
                    if h == 2 and i < 8:
                        dst = idx_scrq[i // 4, :, i % 4, :]
                    else:
                        dst = idx_scr[h, i // 8, :, i % 8, :]
                    nc.sync.dma_start(
                        dst.rearrange("r a -> a r"), idxall[:, i:i + 1])
                    pump(h, p, idxall)
                if h < 2:
                    half_gather(h, idxall, 0)
            quarter_gather(2, idxall, 0)
            proj_tbs([0, 1, 2, 3])

            spl_cm.__exit__(None, None, None)
            xpool_cm.__exit__(None, None, None)


    nc.finalize()
    return nc


_NC_CACHE = {}


def _split16(a):
    hi = a.astype(np.float16)
    lo = (a - hi.astype(np.float32)).astype(np.float16)
    return hi, lo


def make_in_maps(x, w_attn, b_attn, w_proj, b_proj, gumbel):
    B, T_, C_ = x.shape
    assert (B, T_, C_) == (2, T, C)
    assert np.all(b_attn == 0.0), "kernel assumes zero attn bias"
    scale = np.float32(1.0 / np.sqrt(HD))

    jj = np.arange(128)
    mdiag = np.where(jj[None, :] <= jj[:, None], 0.0, NEG).astype(np.float32)
    id16 = np.eye(128, dtype=np.float16)
    iotar = np.broadcast_to(np.arange(T, dtype=np.float16)[None, :],
                            (128, T)).copy()
    repl16 = np.tile(np.eye(16, dtype=np.float32), (1, 8))

    in_maps = []
    for core in range(8):
        b, h0 = core // 4, HPC * (core % 4)
        cq = slice(h0 * HD, (h0 + HPC) * HD)

        xb = x[b]
        xh, xl = _split16(xb)
        xhT = np.ascontiguousarray(xh.T)
        xlT = np.ascontiguousarray(xl.T)

        wq = w_attn[:, cq.start:cq.stop] * scale
        wk = w_attn[:, C + cq.start:C + cq.stop]
        wqk = np.concatenate(
            [np.concatenate([wq[:, h * HD:(h + 1) * HD],
                             wk[:, h * HD:(h + 1) * HD]], axis=1)
             for h in range(HPC)], axis=1)              # [C, 384] per-head q|k
        wqk_h, wqk_l = _split16(wqk)
        wv = w_attn[:, 2 * C + cq.start:2 * C + cq.stop]
        wv_h = wv.astype(np.float16)
        wp16 = w_proj[cq, :].astype(np.float16)

        gmod = np.ascontiguousarray(gumbel[b, h0:h0 + HPC]).copy()
        for i in range(NB):
            s = slice(i * 128, (i + 1) * 128)
            gmod[:, s, s] += mdiag[None]

        in_maps.append({
            "xhT": xhT, "xlT": xlT,
            "wqkh": np.ascontiguousarray(wqk_h),
            "wqkl": np.ascontiguousarray(wqk_l),
            "wvh": np.ascontiguousarray(wv_h),
            "wpd": wp16,
            "gum": gmod,
            "id16": id16,
            "iotar": iotar,
            "repl16": repl16,
        })
    return in_maps


def kernel(x, w_attn, b_attn, w_proj, b_proj, gumbel, _trace=False):
    B = x.shape[0]
    if "nc" not in _NC_CACHE:
        _NC_CACHE["nc"] = build_program()
    nc = _NC_CACHE["nc"]
    in_maps = make_in_maps(x, w_attn, b_attn, w_proj, b_proj, gumbel)

    res = run_bass_kernel_spmd(nc, in_maps, core_ids=list(range(8)), trace=_trace)
    parts = [r["out"].astype(np.float32) for r in res.results]
    outp = np.empty((B, T, C), dtype=np.float32)
    for b in range(B):
        outp[b] = parts[4 * b] + parts[4 * b + 1] + parts[4 * b + 2] + parts[4 * b + 3]
        outp[b] += b_proj[None, :]
    if _trace:
        return outp, res
    return outp


# revision 9
# speedup vs baseline: 1.0165x; 1.0165x over previous
"""Trainium2 Bass kernel for gumbel hard-attention (sparse_attention), v2.

Forward value of hard gumbel-softmax is one_hot(argmax(mask(q@kT)*scale+gum)),
so y[b,h,q,:] = v[b,h,argmax_k,:]; softmax is monotone, argmax over logits.

Precision: the argmax needs ~2^-17 logit fidelity (one flipped row costs
~1e-2 rel err). q,k and q@kT are computed with fp16 hi/lo split arithmetic
(error ~2^-21): x and w_qk are split on the host into fp16 (hi, lo) pairs;
q^T,k^T = 18 accumulated fp16 matmuls (6 hi*hi + 12 cross); the qk matmul
runs as hh (K=64) plus one concatenated cross matmul [qh;ql]@[kl;kh] (K=128).
v and c_proj run in plain fp16.

Per block-row: DVE does a fused (att+gum, running-max) pass via
tensor_tensor_reduce; GpSimd extracts the argmax position with
scalar_tensor_tensor sum((apg==M)*iota). The v-gather is a SWDGE
dma_gather (transpose mode) from a DRAM scratch copy of v, yielding y^T
directly; indices are re-wrapped to the [16]-partition layout via a tiny
DRAM bounce and replicated to 128 partitions with a matmul.

Gumbel streams on the DVE-issued HWDGE queue so it prefetches while the
SP queue loads x/w; x/w/out use the SP queue.

Sharding: 24 (b,h) over 8 cores -> 3 heads/core; partial c_proj summed on
host. Causal mask is folded into the gumbel diagonal blocks host-side.
"""

import sys

for p in ("/opt/trn_rl_repo",):
    if p not in sys.path:
        sys.path.insert(0, p)

import numpy as np

import concourse.bacc as bacc
import concourse.dve_ops as dve_ops
import concourse.mybir as mybir
from concourse.bass_utils import run_bass_kernel_spmd
from concourse.dve_spec import (AluOp, C0, C1, Idx, One, Spec, Src0, Src1,
                                Zero, eq, scan, select)
from concourse.tile import TileContext


def _ref_add_argmax(in0, in1, c0, c1, c2):
    """out = where(z==runmax, idx, -1) candidates of z = in0+in1;
    accum = max(out) = argmax position (last max wins)."""
    z = in0.astype(np.float32) + in1.astype(np.float32)
    r = np.maximum.accumulate(z, axis=-1)
    ii = np.arange(z.shape[-1], dtype=np.float32)[None, :]
    cand = np.where(z == r, np.broadcast_to(ii, z.shape), np.float32(-1.0))
    return cand, cand.max(axis=-1, keepdims=True)


def _register_add_argmax():
    if "ADD_ARGMAX_ANT" in dve_ops._SUB_OPCODE_FOR_NAME:
        return [op for op in dve_ops.OPS if op.name == "ADD_ARGMAX_ANT"][0]
    z = Src0 + Src1
    r = scan(AluOp.MAX, z)
    cand = select(eq(z, r), Idx, Zero - One)
    spec = Spec(body=cand, accum=AluOp.MAX, reference=_ref_add_argmax)
    op = dve_ops.DveOp(
        "ADD_ARGMAX_ANT", spec, subdim=False,
        uops_sha={"v3": "d4bb101012599987"})
    dve_ops.OPS.append(op)
    dve_ops.CUSTOM_DVE_SPECS[op.name] = spec
    dve_ops._SUB_OPCODE_FOR_NAME[op.name] = (
        max(dve_ops._SUB_OPCODE_FOR_NAME.values()) + 1)
    return op


_ADD_ARGMAX = _register_add_argmax()

F32 = mybir.dt.float32
BF16 = mybir.dt.bfloat16
FP16 = mybir.dt.float16
I16 = mybir.dt.int16

T = 2048          # sequence length
C = 768           # model dim
HPC = 3           # heads per core
HD = 64           # head dim
NB = T // 128     # 16 q-blocks
EC = C // 128     # 6 contraction chunks
NEG = -1e9
QKC = 2 * HPC * HD   # 384 = packed q|k head-dim columns
VC = HPC * HD        # 192


def build_program():
    nc = bacc.Bacc(target_bir_lowering=False, trn_type="TRN2")

    xhT = nc.dram_tensor("xhT", [C, T], FP16, kind="ExternalInput")
    xlT = nc.dram_tensor("xlT", [C, T], FP16, kind="ExternalInput")
    wqkh = nc.dram_tensor("wqkh", [C, QKC], FP16, kind="ExternalInput")
    wqkl = nc.dram_tensor("wqkl", [C, QKC], FP16, kind="ExternalInput")
    wvh = nc.dram_tensor("wvh", [C, VC], FP16, kind="ExternalInput")
    wpd = nc.dram_tensor("wpd", [VC, C], FP16, kind="ExternalInput")
    gum = nc.dram_tensor("gum", [HPC, T, T], F32, kind="ExternalInput")
    id16 = nc.dram_tensor("id16", [128, 128], FP16, kind="ExternalInput")
    id32 = nc.dram_tensor("id32", [128, 128], F32, kind="ExternalInput")
    iotar = nc.dram_tensor("iotar", [128, T], FP16, kind="ExternalInput")
    repl16 = nc.dram_tensor("repl16", [16, 128], F32, kind="ExternalInput")

    # v rows in gather layout: row t = [v_h0(64) pad(64) v_h1 pad v_h2 pad]
    v_scr = nc.dram_tensor("v_scr", [NB, 128, HPC, 2, HD], FP16, kind="Internal")
    # wrapped argmax indices per half: [head][half][r][blk'][a]
    # (query q = 1024*half + 128*blk' + 16*a + r)
    idx_scr = nc.dram_tensor("idx_scr", [HPC, 2, 16, 8, 8], F32, kind="Internal")
    # quarter-granularity variant for the final head's tail
    idx_scrq = nc.dram_tensor("idx_scrq", [4, 16, 4, 8], F32, kind="Internal")

    out = nc.dram_tensor("out", [T, C], BF16, kind="ExternalOutput")

    with TileContext(nc) as tc:
        with (
            tc.tile_pool(name="const", bufs=1) as cpool,
            tc.tile_pool(name="big", bufs=1) as bigpool,
            tc.tile_pool(name="gumld", bufs=7) as gpool,
            tc.tile_pool(name="scr", bufs=1) as scrpool,
            tc.tile_pool(name="stat", bufs=4) as spool,
            tc.tile_pool(name="io", bufs=3) as iopool,
            tc.tile_pool(name="psqk", bufs=1, space="PSUM") as pqk_pool,   # 4 banks
            tc.tile_pool(name="psmm", bufs=2, space="PSUM") as pmm,       # 2 banks
        ):
            id16_sb = cpool.tile([128, 128], FP16, tag="id16")
            id32_sb = cpool.tile([128, 128], F32, tag="id32")
            repl_sb = cpool.tile([16, 128], F32, tag="repl16")
            wqkh_sb = bigpool.tile([128, EC, QKC], FP16, tag="wqkh")
            wqkl_sb = bigpool.tile([128, EC, QKC], FP16, tag="wqkl")
            wvh_sb = bigpool.tile([128, EC, VC], FP16, tag="wvh")
            nc.sync.dma_start(wqkh_sb, wqkh.rearrange("(e p) c -> p e c", p=128))
            nc.sync.dma_start(wqkl_sb, wqkl.rearrange("(e p) c -> p e c", p=128))
            wpH = bigpool.tile([64, HPC, C], FP16, tag="wpH")
            wpH2 = bigpool.tile([128, C], FP16, tag="wpH2")

            # ---- Phase 1: q^T/k^T hi+lo stacks and v^T, streaming x by t4
            # qstack: partitions 0-63 = qh^T(head), 64-127 = ql^T(head)
            # kstack: partitions 0-63 = kl^T(head), 64-127 = kh^T(head)
            # khT2:   kh^T duplicated at base partition 0 (hh matmul rhs)
            qstack = bigpool.tile([128, HPC, T], FP16, tag="qstack")
            kstack = bigpool.tile([128, HPC, T], FP16, tag="kstack")
            khT2 = bigpool.tile([64, HPC, T], FP16, tag="khT2")
            vtA = bigpool.tile([128, T], F32, tag="vtA")   # v^T heads 0,1
            vtB = bigpool.tile([64, T], F32, tag="vtB")    # v^T head 2

            xhT_r = xhT.rearrange("(e p) t -> p e t", p=128)
            xlT_r = xlT.rearrange("(e p) t -> p e t", p=128)
            xpool_cm = tc.tile_pool(name="xT", bufs=1)
            xpool = xpool_cm.__enter__()
            xh_sb = xpool.tile([128, EC, T], FP16, tag="xh")
            xl_sb = xpool.tile([128, EC, T], FP16, tag="xl")
            for ec in range(EC):
                nc.sync.dma_start(xh_sb[:, ec, :], xhT_r[:, ec, :])
                nc.sync.dma_start(xl_sb[:, ec, :], xlT_r[:, ec, :])
            nc.sync.dma_start(wvh_sb, wvh.rearrange("(e p) c -> p e c", p=128))
            nc.sync.dma_start(id16_sb, id16[:, :])
            nc.sync.dma_start(id32_sb, id32[:, :])
            nc.sync.dma_start(repl_sb, repl16[:, :])
            nc.sync.dma_start(wpH, wpd.rearrange("(h p) c -> p h c", p=64))
            nc.sync.dma_start(wpH2, wpd[0:128, :])
            spl_cm = tc.tile_pool(name="split", bufs=2)
            spl = spl_cm.__enter__()

            v_nat = bigpool.tile([128, NB, VC], FP16, tag="v_nat")
            ystack = bigpool.tile([128, T], FP16, tag="ystack")  # h0 | h1
            y1 = bigpool.tile([128, T], FP16, tag="y1")
            y2 = bigpool.tile([128, T], FP16, tag="y2")
            v_flat = v_scr.rearrange("b p h s e -> (b p) (h s e)")

            def _splits(h, ts, pq):
                hi16 = spl.tile([128, 512], FP16, tag="hi")
                lo16 = spl.tile([128, 512], FP16, tag="lo")
                nc.vector.tensor_copy(hi16, pq[:, 0:512])
                nc.vector.tensor_tensor(
                    lo16, pq[:, 0:512], hi16, mybir.AluOpType.subtract)
                nc.gpsimd.tensor_copy(qstack[0:64, h, ts], hi16[0:64, :])
                nc.gpsimd.tensor_copy(qstack[64:128, h, ts], lo16[0:64, :])
                nc.gpsimd.tensor_copy(kstack[64:128, h, ts], hi16[64:128, :])
                nc.gpsimd.tensor_copy(khT2[:, h, ts], hi16[64:128, :])
                nc.gpsimd.tensor_copy(kstack[0:64, h, ts], lo16[64:128, :])

            _JWX = ([(0, 0)] * 6 + [(1, 0)] * 6 + [(0, 1)] * 6,)[0]

            def _piece_mm(pq, h, t4, j, j0, j1):
                wi, xi = _JWX[j]
                wsb = (wqkh_sb, wqkl_sb)[wi]
                xsb = (xh_sb, xl_sb)[xi]
                ec = j % 6
                ts = slice(t4 * 512, (t4 + 1) * 512)
                nc.tensor.matmul(
                    pq[:, 0:512],
                    wsb[:, ec, h * 128:(h + 1) * 128],
                    xsb[:, ec, ts],
                    start=(j == j0), stop=(j == j1))

            piece_ps = {}

            def piece_half(h, t4, part):
                """9 of the 18 accumulating qkv matmuls for (head, t4)."""
                ts = slice(t4 * 512, (t4 + 1) * 512)
                if part == 0:
                    piece_ps[(h, t4)] = pmm.tile(
                        [128, 512], F32, tag="mm", name=f"pp_{h}_{t4}")
                pq = piece_ps[(h, t4)]
                for j in range(part * 9, part * 9 + 9):
                    _piece_mm(pq, h, t4, j, 0, 17)
                if part == 1:
                    _splits(h, ts, pq)

            def piece0():
                """Head-0 piece: 4 concurrent ec-outer groups so the matmuls
                track the x chunk arrivals."""
                big = pqk_pool.tile([128, 2048], F32, tag="pqk", name="pgb")
                gps = [pmm.tile([128, 512], F32, tag="mm", name="pg0"),
                       pmm.tile([128, 512], F32, tag="mm", name="pg1"),
                       big[:, 0:512], big[:, 512:1024]]
                combos = ((wqkh_sb, xh_sb), (wqkl_sb, xh_sb), (wqkh_sb, xl_sb))
                for ec in range(EC):
                    for t4 in range(4):
                        ts = slice(t4 * 512, (t4 + 1) * 512)
                        for ci, (wsb, xsb) in enumerate(combos):
                            nc.tensor.matmul(
                                gps[t4][:, 0:512],
                                wsb[:, ec, 0:128],
                                xsb[:, ec, ts],
                                start=(ec == 0 and ci == 0),
                                stop=(ec == EC - 1 and ci == 2))
                for t4 in range(4):
                    _splits(0, slice(t4 * 512, (t4 + 1) * 512), gps[t4])

            def v_group(t4, pc):
                ts = slice(t4 * 512, (t4 + 1) * 512)
                dst, cn = ((vtA, 128), (vtB, 64))[pc]
                pv = pmm.tile([128, 512], F32, tag="mm")
                for ec in range(EC):
                    nc.tensor.matmul(
                        pv[:cn, 0:512],
                        wvh_sb[:, ec, pc * 128:pc * 128 + cn],
                        xh_sb[:, ec, ts],
                        start=(ec == 0), stop=(ec == EC - 1))
                nc.vector.tensor_copy(dst[:, ts], pv[:cn, 0:512])

            def vnat_half(g):
                for tb in range(g * 8, g * 8 + 8):
                    bs = slice(tb * 128, (tb + 1) * 128)
                    pv = pmm.tile([128, 512], F32, tag="mm", name="pvt")
                    nc.tensor.transpose(pv[:, 0:128], vtA[:, bs], id32_sb)
                    nc.tensor.transpose(pv[:, 128:192], vtB[:, bs],
                                        id32_sb[0:64, 0:64])
                    nc.vector.tensor_copy(v_nat[:, tb, :], pv[:, 0:192])

            def vscr_write():
                for hh in range(HPC):
                    nc.sync.dma_start(
                        v_scr[:, :, hh, 0, :].rearrange("b p e -> p b e"),
                        v_nat[:, :, hh * HD:(hh + 1) * HD])

            def half_gather(h, idxall, half):
                """Wrap+gather queries [half*1024, (half+1)*1024)."""
                n2 = T // 2
                rb16 = spool.tile([16, 64], F32, tag="rb16")
                nc.sync.dma_start(
                    rb16, idx_scr[h, half].rearrange("r b a -> r (b a)"))
                pidx = pmm.tile([128, 512], F32, tag="mm")
                nc.tensor.matmul(pidx[:, 0:64], repl_sb, rb16,
                                 start=True, stop=True)
                idx16 = spool.tile([128, 64], I16, tag="idx16")
                nc.vector.tensor_copy(idx16, pidx[:, 0:64])
                ydst = (ystack, y1, y2)[h]
                hsl = slice(half * n2, (half + 1) * n2)
                nc.gpsimd.dma_gather(
                    out_ap=ydst[:, hsl].rearrange("p (s t) -> p s t", s=1),
                    in_ap=v_flat[:, h * 128:(h + 1) * 128],
                    idxs_ap=idx16,
                    num_idxs=n2,
                    num_idxs_reg=n2,
                    elem_size=128,
                    elem_step=2 * HPC * HD,
                    single_packet=False,
                    transpose=True)
                if h == 1:
                    nc.gpsimd.tensor_copy(ystack[64:128, hsl], y1[0:64, hsl])

            def quarter_gather(h, idxall, qt):
                """Wrap+gather queries [qt*512, (qt+1)*512) (h=2 tail)."""
                n4 = T // 4
                rb16 = spool.tile([16, 32], F32, tag="rb16")
                nc.sync.dma_start(
                    rb16, idx_scrq[qt].rearrange("r b a -> r (b a)"))
                pidx = pmm.tile([128, 512], F32, tag="mm")
                nc.tensor.matmul(pidx[:, 0:32], repl_sb, rb16,
                                 start=True, stop=True)
                idx16 = spool.tile([128, 32], I16, tag="idx16")
                nc.vector.tensor_copy(idx16, pidx[:, 0:32])
                ydst = (ystack, y1, y2)[h]
                hsl = slice(qt * n4, (qt + 1) * n4)
                nc.gpsimd.dma_gather(
                    out_ap=ydst[:, hsl].rearrange("p (s t) -> p s t", s=1),
                    in_ap=v_flat[:, h * 128:(h + 1) * 128],
                    idxs_ap=idx16,
                    num_idxs=n4,
                    num_idxs_reg=n4,
                    elem_size=128,
                    elem_step=2 * HPC * HD,
                    single_packet=False,
                    transpose=True)

            def proj_tbs(tbs):
                for tb in tbs:
                    bs = slice(tb * 128, (tb + 1) * 128)
                    ost = iopool.tile([128, C], BF16, tag="ost")
                    for fc in (0, 384):
                        po = pmm.tile([128, 512], F32, tag="mm", name="po")
                        nc.tensor.matmul(
                            po[:, 0:384], ystack[:, bs],
                            wpH2[:, fc:fc + 384], start=True, stop=False)
                        nc.tensor.matmul(
                            po[:, 0:384], y2[0:64, bs],
                            wpH[:, 2, fc:fc + 384], start=False, stop=True)
                        nc.scalar.copy(ost[:, fc:fc + 384], po[:, 0:384])
                    nc.sync.dma_start(out[bs, :], ost)

            def pump(h, p, idxall):
                """Emit background PE work after head h's p-th processed
                block (blocks run big-to-small: block = 15 - p)."""
                if h == 0:
                    if p < 8:
                        v_group(p // 2, p % 2)
                    if 4 <= p < 12:
                        piece_half(1, (p - 4) // 2, (p - 4) % 2)
                    if p == 12:
                        vnat_half(0)
                    elif p == 13:
                        vnat_half(1)
                    elif p == 14:
                        vscr_write()
                    elif p == 15:
                        half_gather(0, idxall, 1)
                elif h == 1:
                    if p < 8:
                        piece_half(2, p // 2, p % 2)
                    if p == 8:
                        half_gather(1, idxall, 1)
                else:
                    if p == 7:
                        half_gather(2, idxall, 1)
                    elif 8 <= p < 12:
                        proj_tbs([p])
                    elif p == 12:
                        quarter_gather(2, idxall, 1)
                        proj_tbs([12])
                    elif p in (13, 14):
                        proj_tbs([p, p - 9])
                    elif p == 15:
                        proj_tbs([15, 6, 7])

            piece0()
            for h in range(HPC):
                idxall = spool.tile([128, NB], F32, tag="idxall")
                for p in range(NB):
                    i = NB - 1 - p
                    W = (i + 1) * 128
                    qs = slice(i * 128, (i + 1) * 128)
                    gt = gpool.tile([128, T], F32, tag="gum")
                    nc.scalar.dma_start(gt[:, :W], gum[h, qs, 0:W])
                    pa = pqk_pool.tile([128, 2048], F32, tag="pqk")
                    for s in range(0, W, 512):
                        sw = min(512, W - s)
                        ks = slice(s, s + sw)
                        nc.tensor.matmul(
                            pa[:, s:s + sw],
                            qstack[0:64, h, qs], khT2[:, h, ks],
                            start=True, stop=False)
                        nc.tensor.matmul(
                            pa[:, s:s + sw],
                            qstack[:, h, qs], kstack[:, h, ks],
                            start=False, stop=True)
                    scr = scrpool.tile([128, T], FP16, tag="scr")
                    nc.vector._custom_dve(
                        _ADD_ARGMAX,
                        out=scr[:, :W],
                        in0=pa[:, :W],
                        in1=gt[:, :W],
                        accum_out=idxall[:, i:i + 1])
                    if h == 2 and i < 8:
                        dst = idx_scrq[i // 4, :, i % 4, :]
                    else:
                        dst = idx_scr[h, i // 8, :, i % 8, :]
                    nc.sync.dma_start(
                        dst.rearrange("r a -> a r"), idxall[:, i:i + 1])
                    pump(h, p, idxall)
                if h < 2:
                    half_gather(h, idxall, 0)
            quarter_gather(2, idxall, 0)
            proj_tbs([0, 1, 2, 3])

            spl_cm.__exit__(None, None, None)
            xpool_cm.__exit__(None, None, None)


    nc.finalize()
    return nc


_NC_CACHE = {}


def _split16(a):
    hi = a.astype(np.float16)
    lo = (a - hi.astype(np.float32)).astype(np.float16)
    return hi, lo


def make_in_maps(x, w_attn, b_attn, w_proj, b_proj, gumbel):
    B, T_, C_ = x.shape
    assert (B, T_, C_) == (2, T, C)
    assert np.all(b_attn == 0.0), "kernel assumes zero attn bias"
    scale = np.float32(1.0 / np.sqrt(HD))

    jj = np.arange(128)
    mdiag = np.where(jj[None, :] <= jj[:, None], 0.0, NEG).astype(np.float32)
    id16 = np.eye(128, dtype=np.float16)
    iotar = np.broadcast_to(np.arange(T, dtype=np.float16)[None, :],
                            (128, T)).copy()
    repl16 = np.tile(np.eye(16, dtype=np.float32), (1, 8))

    in_maps = []
    for core in range(8):
        b, h0 = core // 4, HPC * (core % 4)
        cq = slice(h0 * HD, (h0 + HPC) * HD)

        xb = x[b]
        xh, xl = _split16(xb)
        xhT = np.ascontiguousarray(xh.T)
        xlT = np.ascontiguousarray(xl.T)

        wq = w_attn[:, cq.start:cq.stop] * scale
        wk = w_attn[:, C + cq.start:C + cq.stop]
        wqk = np.concatenate(
            [np.concatenate([wq[:, h * HD:(h + 1) * HD],
                             wk[:, h * HD:(h + 1) * HD]], axis=1)
             for h in range(HPC)], axis=1)              # [C, 384] per-head q|k
        wqk_h, wqk_l = _split16(wqk)
        wv = w_attn[:, 2 * C + cq.start:2 * C + cq.stop]
        wv_h = wv.astype(np.float16)
        wp16 = w_proj[cq, :].astype(np.float16)

        gmod = np.ascontiguousarray(gumbel[b, h0:h0 + HPC]).copy()
        for i in range(NB):
            s = slice(i * 128, (i + 1) * 128)
            gmod[:, s, s] += mdiag[None]

        in_maps.append({
            "id32": np.eye(128, dtype=np.float32),
            "xhT": xhT, "xlT": xlT,
            "wqkh": np.ascontiguousarray(wqk_h),
            "wqkl": np.ascontiguousarray(wqk_l),
            "wvh": np.ascontiguousarray(wv_h),
            "wpd": wp16,
            "gum": gmod,
            "id16": id16,
            "iotar": iotar,
            "repl16": repl16,
        })
    return in_maps


def kernel(x, w_attn, b_attn, w_proj, b_proj, gumbel, _trace=False):
    B = x.shape[0]
    if "nc" not in _NC_CACHE:
        _NC_CACHE["nc"] = build_program()
    nc = _NC_CACHE["nc"]
    in_maps = make_in_maps(x, w_attn, b_attn, w_proj, b_proj, gumbel)

    res = run_bass_kernel_spmd(nc, in_maps, core_ids=list(range(8)), trace=_trace)
    parts = [r["out"].astype(np.float32) for r in res.results]
    outp = np.empty((B, T, C), dtype=np.float32)
    for b in range(B):
        outp[b] = parts[4 * b] + parts[4 * b + 1] + parts[4 * b + 2] + parts[4 * b + 3]
        outp[b] += b_proj[None, :]
    if _trace:
        return outp, res
    return outp
